# revision 24
# baseline (speedup 1.0000x reference)
"""DolmaGPT (4-layer GPT, D=1024, H=16, T=1024, B=2, V=32000, ALiBi) on 8 TRN2 cores.

Strategy: sequence-parallel. Each core owns 256 token rows (cores 0-3 batch 0,
cores 4-7 batch 1). Weights replicated (bf16, streamed from HBM in pre-tiled
layouts so each load is one large-descriptor DMA). Per layer one fused 4-core
AllGather exchanges K^T and V together (bf16). lm_head vocab-sharded after an
8-core AllGather of the final hidden state. Residual stream fp32 in SBUF;
matmuls bf16 with fp32 PSUM accumulation.

v2 notes (instruction-count + pipelining oriented):
- Weights pre-tiled on host to [P, KC, F] so each load is one DMA with 2KB
  descriptors (v1 used 16-32 small strided DMAs per weight at 256B/desc).
- LN transposes via the DMA XBAR (dma_start_transpose) instead of PE
  transpose + copy chains.
- PSUM tiles span 4 banks' worth of chunks ([P, 4, NTOK]) so exp / gelu /
  PSUM->SBUF copies are one instruction per 4 matmul groups.
- K and V share one AllGather message per layer.
- lm_head processes vocab chunks in pairs per PSUM tile; output stored bf16.
- Softmax denominator broadcast on gpsimd (partition_broadcast), psy PSUM
  double-buffered: successive attention heads pipeline (this was worth ~2x
  on measured HW time).
- Output stores on the Act HWDGE queue. NOTE: gpsimd/SWDGE dma_start stores
  crashed the device (NRT_EXEC_UNIT_UNRECOVERABLE) alongside collectives.

Softmax: scores bounded, so no max-subtract. P = exp(scale*s) * M where
M = exp(alibi_bias) (0 where masked) is a precomputed per-core constant.
Denominator via ones-column appended to V (one extra PSUM row per head).
"""
import contextlib
import math
import numpy as np
import ml_dtypes

import concourse.bacc as bacc
import concourse.bass as bass
import concourse.mybir as mybir
import concourse.tile as tile
from concourse.bass import ts, ds
from concourse.masks import make_identity

P = 128
HD = 64
EPS = 1e-5
ALIBI_BIAS_MAX = 8.0
NCORES = 8
GS = 4  # AllGather group size for K/V (cores sharing one batch element)

FULL = dict(V=32000, D=1024, H=16, L=4, F=4096, B=2, T=1024,
            store_act=True)

F32 = mybir.dt.float32
BF16 = mybir.dt.bfloat16
I32 = mybir.dt.int32


def build_program(cfg):
    V, D, H, L, F, B, T = (cfg[k] for k in ("V", "D", "H", "L", "F", "B", "T"))
    NTOK = B * T // NCORES      # tokens per core
    QT = NTOK // P              # q-token tiles per core
    KC = D // P                 # contract chunks over D
    KT = T // P                 # k-token tiles (attention keys, own batch)
    FT = F // P                 # MLP hidden tiles
    FC = F // P                 # fc2 contract chunks
    VS = V // NCORES            # vocab shard per core
    NV = 500 if VS % 500 == 0 else (128 if VS % 128 == 0 else VS)
    NVC = VS // NV
    MT = B * T // P             # global token tiles (lm_head rows)
    scale = 1.0 / math.sqrt(HD)
    NCH = min(512, D)           # N-chunk for [tok, feat] matmuls
    WF = min(1024, D)           # feature width of one weight tile
    XX = D // NTOK if D >= NTOK else 0   # v-export row split (see kvmsg)
    H2 = NTOK // HD             # heads per 256-col row-chunk of v region
    ZB = bool(cfg.get("zero_bias"))  # skip K=1 bias matmuls when biases zero
    STORE_ACT = bool(cfg.get("store_act"))   # osb stores via Act HWDGE
    NO_XBAR = bool(cfg.get("no_xbar"))       # PE transposes instead of XBAR
    SP_DMA = bool(cfg.get("sp_dma"))         # all loads via SP queue
    Q4 = 4                      # chunk group per PSUM tile

    assert D % NTOK == 0 and H == (D // NTOK) * (NTOK // HD)
    assert H % 4 == 0 or H == 4

    nc = bacc.Bacc("TRN2", target_bir_lowering=False, debug=False,
                   num_devices=NCORES)
    eng2 = nc.sync if SP_DMA else nc.scalar   # gathered acts / masks queue

    # ---- DRAM parameters (identical shapes on every core) ----
    # pre-tiled weights: [.., P, kc, feat] so one load = one big-desc DMA
    ids_in = nc.declare_dram_parameter("ids", [QT, P], I32, isOutput=False)
    wte_in = nc.declare_dram_parameter("wte", [V, D], F32, isOutput=False)
    # slot 0 = K feats, slot 1 = V feats, slot 2 = Q feats
    wkvq_in = nc.declare_dram_parameter("wkvq", [L, 3, P, KC, D], BF16,
                                        isOutput=False)
    wproj_in = nc.declare_dram_parameter("wprojT", [L, P, KC, D], BF16,
                                         isOutput=False)
    wfc_in = nc.declare_dram_parameter("wfcT", [L, P, KC, F], BF16,
                                       isOutput=False)
    wfc2_in = nc.declare_dram_parameter("wfc2T", [L, P, FC, D], BF16,
                                        isOutput=False)
    msk_in = nc.declare_dram_parameter("msk", [max(1, H // 4), T, min(H, 4), NTOK],
                                       BF16, isOutput=False)
    wlm_in = nc.declare_dram_parameter("wlmT", [P, KC, VS], BF16, isOutput=False)
    if not ZB:
        qkb_in = nc.declare_dram_parameter("qkb", [L, 2 * D], F32, isOutput=False)
        vb_in = nc.declare_dram_parameter("vb", [L, D], F32, isOutput=False)
        pb_in = nc.declare_dram_parameter("pb", [L, D], F32, isOutput=False)
        fcb_in = nc.declare_dram_parameter("fcb", [L, F], F32, isOutput=False)
        f2b_in = nc.declare_dram_parameter("f2b", [L, D], F32, isOutput=False)
        lmb_in = nc.declare_dram_parameter("lmb", [VS], F32, isOutput=False)
    out_d = nc.declare_dram_parameter("out", [B * T, VS], BF16, isOutput=True)

    HG = min(H, 4)              # heads per mask tile
    NHG = H // HG

    with tile.TileContext(nc) as tc:
        with (
            tc.tile_pool(name="const", bufs=1) as constp,
            tc.tile_pool(name="resident", bufs=1) as resp,
            tc.tile_pool(name="acts", bufs=1) as actp,
            tc.tile_pool(name="w", bufs=3) as wp_,
            tc.tile_pool(name="ln", bufs=2) as lnp,
            tc.tile_pool(name="stats", bufs=4) as statp,
            tc.tile_pool(name="mask", bufs=2) as maskp,
            tc.tile_pool(name="softmax", bufs=2) as softp,
            tc.tile_pool(name="outcp", bufs=2) as outp,
            tc.tile_pool(name="psA", bufs=(2 if NO_XBAR else 3),
                         space="PSUM") as psA,
            tc.tile_pool(name="psY", bufs=2, space="PSUM") as psY,
            tc.tile_pool(name="dram", bufs=2, space="DRAM") as dramp,
            contextlib.ExitStack() as estack,
        ):
            psT = (estack.enter_context(
                tc.tile_pool(name="psT", bufs=2, space="PSUM"))
                if NO_XBAR else None)
            # ---- constants ----
            if NO_XBAR:
                id_bf = constp.tile([P, P], BF16)
                make_identity(nc, id_bf[:, :])
            ones_bank = constp.tile([65, P], F32)
            nc.vector.memset(ones_bank[:, :], 1.0)
            eps_sb = constp.tile([P, 1], F32)
            nc.vector.memset(eps_sb[:, :], EPS)

            if not ZB:
                QKC = 2 * D // P
                qkb_sb = constp.tile([P, L * QKC], F32)
                nc.scalar.dma_start(
                    out=qkb_sb[:, :],
                    in_=qkb_in[:, :].rearrange("l (c p) -> p (l c)", p=P))
                fcb_sb = constp.tile([P, L * FT], F32)
                nc.scalar.dma_start(
                    out=fcb_sb[:, :],
                    in_=fcb_in[:, :].rearrange("l (c p) -> p (l c)", p=P))
                rows_sb = constp.tile([65, L * D], F32)
                nc.scalar.dma_start(
                    out=rows_sb[0:1, 0:L * D],
                    in_=vb_in[:, :].rearrange("l d -> (l d)")
                    .rearrange("(o f) -> o f", o=1))
                nc.scalar.dma_start(
                    out=rows_sb[32:33, 0:L * D],
                    in_=pb_in[:, :].rearrange("l d -> (l d)")
                    .rearrange("(o f) -> o f", o=1))
                nc.scalar.dma_start(
                    out=rows_sb[64:65, 0:L * D],
                    in_=f2b_in[:, :].rearrange("l d -> (l d)")
                    .rearrange("(o f) -> o f", o=1))
                vb_sb = rows_sb[0:1, :]
                pb_sb = rows_sb[32:33, :]
                f2b_sb = rows_sb[64:65, :]

            # ---- residual stream x [p, a, d], fp32, resident ----
            x = resp.tile([P, QT, D], F32)
            for a in range(QT):
                ids_sb = statp.tile([P, 1], I32, tag="ids")
                nc.sync.dma_start(out=ids_sb[:, :],
                                  in_=ids_in[a, :].rearrange("(p o) -> p o", o=1))
                nc.gpsimd.indirect_dma_start(
                    out=x[:, a, :], out_offset=None,
                    in_=wte_in[:, :],
                    in_offset=bass.IndirectOffsetOnAxis(ap=ids_sb[:, 0:1], axis=0))

            # ---- v_aug resident [p, kt, h*(HD+1)+j]; ones col per head ----
            v_aug = resp.tile([P, KT, H * (HD + 1)], BF16)
            va4 = v_aug.rearrange("p k (h j) -> p k h j", j=HD + 1)
            nc.vector.memset(va4[:, :, :, HD:HD + 1], 1.0)

            # lm_head gathered hidden state, resident
            xfg_sb = resp.tile([P, NCORES * KC, NTOK], BF16)

            def layer_norm_T(src, zT):
                """LN over free dim of src [P, QT, D] -> transposed bf16 zT
                [P, KC, NTOK] via the DMA XBAR."""
                for a in range(QT):
                    xs = src[:, a, :]
                    sm = statp.tile([P, 1], F32, tag="lnsm")
                    scr = lnp.tile([P, D], BF16, tag="lnscr", bufs=2)
                    nc.scalar.activation(scr[:, :], xs,
                                         mybir.ActivationFunctionType.Identity,
                                         accum_out=sm[:, :])
                    sqd = lnp.tile([P, D], BF16, tag="lnsqd", bufs=2)
                    sx2 = statp.tile([P, 1], F32, tag="lnsx2")
                    nc.scalar.activation(sqd[:, :], xs,
                                         mybir.ActivationFunctionType.Square,
                                         accum_out=sx2[:, :])
                    mean = statp.tile([P, 1], F32, tag="lnmean")
                    nc.scalar.mul(mean[:, :], sm[:, :], 1.0 / D)
                    nmean = statp.tile([P, 1], F32, tag="lnnmean")
                    nc.scalar.mul(nmean[:, :], sm[:, :], -1.0 / D)
                    b2 = statp.tile([P, 1], F32, tag="lnb2")
                    nc.vector.tensor_scalar(out=b2[:, :], in0=mean[:, :],
                                            scalar1=nmean[:, :],
                                            scalar2=eps_sb[:, :],
                                            op0=mybir.AluOpType.mult,
                                            op1=mybir.AluOpType.add)
                    std = statp.tile([P, 1], F32, tag="lnstd")
                    nc.scalar.activation(std[:, :], sx2[:, :],
                                         mybir.ActivationFunctionType.Sqrt,
                                         bias=b2[:, :], scale=1.0 / D)
                    rstd = statp.tile([P, 1], F32, tag="lnrstd")
                    nc.vector.reciprocal(rstd[:, :], std[:, :])
                    mrstd = statp.tile([P, 1], F32, tag="lnmrstd")
                    nc.vector.tensor_mul(out=mrstd[:, :], in0=mean[:, :],
                                         in1=rstd[:, :])
                    z = lnp.tile([P, D], BF16, tag="lnz", bufs=2)
                    nc.vector.tensor_scalar(out=z[:, :], in0=xs,
                                            scalar1=rstd[:, :],
                                            scalar2=mrstd[:, :],
                                            op0=mybir.AluOpType.mult,
                                            op1=mybir.AluOpType.subtract)
                    # zT[p, kc, t] = z[t, kc*128+p]
                    if NO_XBAR:
                        for kc in range(KC):
                            pt = psT.tile([P, P], BF16, tag="tr", bufs=2)
                            nc.tensor.transpose(out=pt[:, :],
                                                in_=z[:, ts(kc, P)],
                                                identity=id_bf[:, :])
                            nc.scalar.copy(out=zT[:, kc, ts(a, P)],
                                           in_=pt[:, :])
                    else:
                        nc.sync.dma_start_transpose(out=zT[:, :, ts(a, P)],
                                                    in_=z[:, :])

            def load_w(src_ap, name):
                """One weight tile [P, KC, <=WF] from a pre-tiled layout.
                All tiles share one uniformly-shaped pool slot."""
                wt = wp_.tile([P, KC, WF], BF16, tag="w", bufs=3, name=name)
                nc.sync.dma_start(out=wt[:, :, 0:src_ap.shape[2]], in_=src_ap)
                return wt

            def mm_quad(dst, wtile, h_src, c0, ncnk, l, bias_sb=None, boff=0,
                        act=None):
                """ncnk feature chunks [feat,tok] into one PSUM tile, then one
                batched copy/activation to dst[:, c0:c0+ncnk, :]."""
                ps = psA.tile([P, Q4, NTOK], F32, tag="quad")
                for j in range(ncnk):
                    for kc in range(KC):
                        nc.tensor.matmul(out=ps[:, j, :],
                                         lhsT=wtile[:, kc, ts(c0 + j, P)],
                                         rhs=h_src[:, kc, :],
                                         start=(kc == 0), stop=(kc == KC - 1))
                fn = act or mybir.ActivationFunctionType.Identity
                if ZB or bias_sb is None:
                    if act is None:
                        nc.scalar.copy(out=dst[:, c0:c0 + ncnk, :],
                                       in_=ps[:, 0:ncnk, :])
                    else:
                        nc.scalar.activation(dst[:, c0:c0 + ncnk, :],
                                             ps[:, 0:ncnk, :], fn)
                else:
                    for j in range(ncnk):
                        nc.scalar.activation(
                            dst[:, c0 + j, :], ps[:, j, :], fn,
                            bias=bias_sb[:, boff + c0 + j: boff + c0 + j + 1])

            for l in range(L):
                # ===== LN1 -> h1T =====
                h1T = actp.tile([P, KC, NTOK], BF16, tag="hT", bufs=1)
                layer_norm_T(x, h1T)

                wk = load_w(wkvq_in[l, 0, :, :, :], f"wk{l}")
                wv = load_w(wkvq_in[l, 1, :, :, :], f"wv{l}")
                wq = load_w(wkvq_in[l, 2, :, :, :], f"wq{l}")

                # ===== kT_loc [feat, tok] =====
                kT = actp.tile([P, KC, NTOK], BF16, tag="kT", bufs=1)
                for c0 in range(0, KC, Q4):
                    n = min(Q4, KC - c0)
                    mm_quad(kT, wk, h1T, c0, n, l,
                            bias_sb=None if ZB else qkb_sb,
                            boff=None if ZB else l * (2 * D // P) + KC)
                # ===== v_sb [tok, feat] (lhsT reused across n0 chunks) =====
                v_sb = actp.tile([P, QT, D], BF16, tag="v_sb", bufs=1)
                nn0 = D // NCH
                AOFF = max(NCH, 512)  # bank-separate concurrent accum groups
                for a in range(QT):
                    ps = psA.tile([P, Q4, NTOK], F32, tag="quad")
                    psf = ps.rearrange("p a q -> p (a q)")
                    for kc in range(KC):
                        for n0 in range(nn0):
                            nc.tensor.matmul(
                                out=psf[:, n0 * AOFF:n0 * AOFF + NCH],
                                lhsT=h1T[:, kc, ts(a, P)],
                                rhs=wv[:, kc, ts(n0, NCH)],
                                start=(kc == 0),
                                stop=(ZB and kc == KC - 1))
                    if not ZB:
                        for n0 in range(nn0):
                            nc.tensor.matmul(
                                out=psf[:, n0 * AOFF:n0 * AOFF + NCH],
                                lhsT=ones_bank[0:1, 0:P],
                                rhs=vb_sb[:, l * D + n0 * NCH:
                                          l * D + n0 * NCH + NCH],
                                start=False, stop=True)
                    if AOFF == NCH:
                        nc.scalar.copy(out=v_sb[:, a, :], in_=psf[:, 0:D])
                    else:
                        for n0 in range(nn0):
                            nc.scalar.copy(out=v_sb[:, a, ts(n0, NCH)],
                                           in_=psf[:, n0 * AOFF:n0 * AOFF + NCH])

                # ===== fused K+V export + AllGather =====
                kvmsg = dramp.tile([2 * D, NTOK], BF16, tag="kvmsg")
                nc.sync.dma_start(
                    out=kvmsg[0:D, :].rearrange("(kc p) q -> p kc q", p=P),
                    in_=kT[:, :, :])
                nc.sync.dma_start(
                    out=kvmsg[D:2 * D, :].rearrange("(a p xx) q -> p a (xx q)",
                                                    p=P, a=QT),
                    in_=v_sb[:, :, :])
                kvg = dramp.tile([GS * 2 * D, NTOK], BF16, tag="kvg")
                if cfg.get("no_cc"):
                    for g in range(GS):
                        nc.sync.dma_start(out=kvg[ts(g, 2 * D), :],
                                          in_=kvmsg[:, :])
                else:
                    nc.gpsimd.collective_compute(
                        "AllGather", mybir.AluOpType.bypass,
                        ins=[kvmsg[:, :].opt()], outs=[kvg[:, :].opt()],
                        replica_groups=[[0, 1, 2, 3], [4, 5, 6, 7]])

                # ===== qT (overlaps the AllGather) =====
                qT = actp.tile([P, KC, NTOK], BF16, tag="qT", bufs=1)
                for c0 in range(0, KC, Q4):
                    n = min(Q4, KC - c0)
                    mm_quad(qT, wq, h1T, c0, n, l,
                            bias_sb=None if ZB else qkb_sb,
                            boff=None if ZB else l * (2 * D // P))

                # ===== unpack gathered K / V =====
                ksb = actp.tile([P, GS * KC, NTOK], BF16, tag="big32", bufs=1)
                for g in range(GS):
                    eng2.dma_start(
                        out=ksb[:, ts(g, KC), :],
                        in_=kvg[g * 2 * D: g * 2 * D + D, :]
                        .rearrange("(kc p) q -> p kc q", p=P))
                    XX1 = max(XX, 1)
                    for k in range(QT):
                        r0 = g * 2 * D + D + k * P * XX1
                        eng2.dma_start(
                            out=va4[:, g * QT + k, :, 0:HD],
                            in_=kvg[r0:r0 + P * XX1, :]
                            .rearrange("(p xx) (h2 j) -> p (xx h2) j",
                                       p=P, j=HD))

                wproj = load_w(wproj_in[l, :, :, :], f"wp{l}")

                # ===== attention per head =====
                yT = actp.tile([P, KC, NTOK], BF16, tag="yT", bufs=1)
                for h in range(H):
                    hr = (h % 2) * HD
                    hc = h // 2
                    hg, h4 = h // HG, h % HG
                    if h4 == 0:
                        mskh = maskp.tile([P, KT, HG, NTOK], BF16, tag="mskh",
                                          bufs=2)
                        eng2.dma_start(
                            out=mskh[:, :, :, :],
                            in_=msk_in[hg, :, :, :]
                            .rearrange("(kc p) g q -> p kc g q", p=P))
                    p_all = softp.tile([P, KT, NTOK], BF16, tag="p_all", bufs=2)
                    for j0 in range(0, KT, Q4):
                        nq = min(Q4, KT - j0)
                        pss = psA.tile([P, Q4, NTOK], F32, tag="quad")
                        for jj in range(nq):
                            jk = j0 + jj
                            g, i = jk // QT, jk % QT
                            nc.tensor.matmul(
                                out=pss[:, jj, :],
                                lhsT=ksb[hr:hr + HD, g * KC + hc, ts(i, P)],
                                rhs=qT[hr:hr + HD, hc, :],
                                start=True, stop=True)
                        esb = softp.tile([P, Q4, NTOK], BF16, tag="esb", bufs=2)
                        nc.scalar.activation(esb[:, 0:nq, :], pss[:, 0:nq, :],
                                             mybir.ActivationFunctionType.Exp,
                                             scale=scale)
                        nc.vector.tensor_mul(out=p_all[:, j0:j0 + nq, :],
                                             in0=esb[:, 0:nq, :],
                                             in1=mskh[:, j0:j0 + nq, h4, :])
                    psy = psY.tile([HD + 1, NTOK], F32, tag="psy", bufs=2)
                    for jk in range(KT):
                        nc.tensor.matmul(
                            out=psy[:, :],
                            lhsT=v_aug[:, jk, h * (HD + 1):(h + 1) * (HD + 1)],
                            rhs=p_all[:, jk, :],
                            start=(jk == 0), stop=(jk == KT - 1))
                    rden = statp.tile([1, NTOK], F32, tag="rden")
                    nc.vector.reciprocal(rden[:, :], psy[HD:HD + 1, :])
                    rbc = softp.tile([HD, NTOK], F32, tag="rbc", bufs=2)
                    nc.gpsimd.partition_broadcast(rbc[:, :], rden[:, :],
                                                  channels=HD)
                    nc.vector.tensor_mul(out=yT[hr:hr + HD, hc, :],
                                         in0=psy[0:HD, :], in1=rbc[:, :])

                # ===== proj + residual (lhsT reused across n0 chunks) =====
                for a in range(QT):
                    ps = psA.tile([P, Q4, NTOK], F32, tag="quad")
                    psf = ps.rearrange("p a q -> p (a q)")
                    for kc in range(KC):
                        for n0 in range(nn0):
                            nc.tensor.matmul(
                                out=psf[:, n0 * AOFF:n0 * AOFF + NCH],
                                lhsT=yT[:, kc, ts(a, P)],
                                rhs=wproj[:, kc, ts(n0, NCH)],
                                start=(kc == 0),
                                stop=(ZB and kc == KC - 1))
                    if not ZB:
                        for n0 in range(nn0):
                            nc.tensor.matmul(
                                out=psf[:, n0 * AOFF:n0 * AOFF + NCH],
                                lhsT=ones_bank[32:33, 0:P],
                                rhs=pb_sb[:, l * D + n0 * NCH:
                                          l * D + n0 * NCH + NCH],
                                start=False, stop=True)
                    if AOFF == NCH:
                        nc.vector.tensor_add(out=x[:, a, :], in0=x[:, a, :],
                                             in1=psf[:, 0:D])
                    else:
                        for n0 in range(nn0):
                            nc.vector.tensor_add(
                                out=x[:, a, ts(n0, NCH)],
                                in0=x[:, a, ts(n0, NCH)],
                                in1=psf[:, n0 * AOFF:n0 * AOFF + NCH])

                # ===== LN2 -> h2T; MLP =====
                h2T = actp.tile([P, KC, NTOK], BF16, tag="hT", bufs=1)
                layer_norm_T(x, h2T)

                nwf = (F + WF - 1) // WF  # fc weight tiles (feature-split)
                wfcs = [load_w(wfc_in[l, :, :, ts(i, WF)], f"wfc{l}_{i}")
                        for i in range(nwf)]
                gT = actp.tile([P, FT, NTOK], BF16, tag="big32", bufs=1)
                FPW = WF // P  # feature chunks per fc tile
                FQ = min(Q4, FPW)
                for c0 in range(0, FT, FQ):
                    n = min(FQ, FT - c0)
                    wt = wfcs[c0 // FPW]
                    ps = psA.tile([P, Q4, NTOK], F32, tag="quad")
                    for j in range(n):
                        loc = (c0 + j) - (c0 // FPW) * FPW
                        for kc in range(KC):
                            nc.tensor.matmul(out=ps[:, j, :],
                                             lhsT=wt[:, kc, ts(loc, P)],
                                             rhs=h2T[:, kc, :],
                                             start=(kc == 0),
                                             stop=(kc == KC - 1))
                    if ZB:
                        nc.scalar.activation(gT[:, c0:c0 + n, :], ps[:, 0:n, :],
                                             mybir.ActivationFunctionType.Gelu)
                    else:
                        for j in range(n):
                            nc.scalar.activation(
                                gT[:, c0 + j, :], ps[:, j, :],
                                mybir.ActivationFunctionType.Gelu,
                                bias=fcb_sb[:, l * FT + c0 + j:
                                            l * FT + c0 + j + 1])

                # ===== fc2: accumulate over FC chunks (lhsT reused over n0) =====
                FB = KC if FC % KC == 0 else min(8, FC)
                w2s = [load_w(wfc2_in[l, :, ts(i, FB), :], f"wfc2{l}_{i}")
                       for i in range(FC // FB)]
                q4s = [psA.tile([P, Q4, NTOK], F32, tag="quad",
                                name=f"fc2q{l}_{a}") for a in range(QT)]
                for fb in range(FC // FB):
                    w2 = w2s[fb]
                    for a in range(QT):
                        q4f = q4s[a].rearrange("p a q -> p (a q)")
                        for j in range(FB):
                            for n0 in range(nn0):
                                nc.tensor.matmul(
                                    out=q4f[:, n0 * AOFF:n0 * AOFF + NCH],
                                    lhsT=gT[:, fb * FB + j, ts(a, P)],
                                    rhs=w2[:, j, ts(n0, NCH)],
                                    start=(fb == 0 and j == 0),
                                    stop=(ZB and fb == FC // FB - 1
                                          and j == FB - 1))
                for a in range(QT):
                    q4f = q4s[a].rearrange("p a q -> p (a q)")
                    if not ZB:
                        for n0 in range(nn0):
                            nc.tensor.matmul(
                                out=q4f[:, n0 * AOFF:n0 * AOFF + NCH],
                                lhsT=ones_bank[64:65, 0:P],
                                rhs=f2b_sb[:, l * D + n0 * NCH:
                                           l * D + n0 * NCH + NCH],
                                start=False, stop=True)
                    if AOFF == NCH:
                        nc.vector.tensor_add(out=x[:, a, :], in0=x[:, a, :],
                                             in1=q4f[:, 0:D])
                    else:
                        for n0 in range(nn0):
                            nc.vector.tensor_add(
                                out=x[:, a, ts(n0, NCH)],
                                in0=x[:, a, ts(n0, NCH)],
                                in1=q4f[:, n0 * AOFF:n0 * AOFF + NCH])

            # ===== final LN -> xfT; AllGather over all 8 cores =====
            wl0 = load_w(wlm_in[:, :, ts(0, NV)], "wl0")
            xfT = actp.tile([P, KC, NTOK], BF16, tag="hT", bufs=1)
            layer_norm_T(x, xfT)
            xfmsg = dramp.tile([D, NTOK], BF16, tag="xfmsg")
            nc.sync.dma_start(
                out=xfmsg[:, :].rearrange("(kc p) q -> p kc q", p=P),
                in_=xfT[:, :, :])
            if cfg.get("no_cc"):
                xfg = dramp.tile([NCORES * D, NTOK], BF16, tag="xfg")
                for g in range(NCORES):
                    nc.sync.dma_start(out=xfg[ts(g, D), :], in_=xfmsg[:, :])
            else:
                xfg = dramp.tile([NCORES * D, NTOK], BF16, tag="xfg",
                                 addr_space="Shared")
                nc.gpsimd.collective_compute(
                    "AllGather", mybir.AluOpType.bypass,
                    ins=[xfmsg[:, :].opt()], outs=[xfg[:, :].opt()],
                    replica_groups=[list(range(NCORES))])
            eng2.dma_start(
                out=xfg_sb[:, :, :],
                in_=xfg[:, :].rearrange("(c p) t -> p c t", p=P))

            # ===== lm_head: all tokens x vocab shard, paired NV chunks =====
            PAIR = 2 if NVC % 2 == 0 else 1
            for np0 in range(0, NVC, PAIR):
                wls = []
                for s in range(PAIR):
                    nn = np0 + s
                    wls.append(wl0 if nn == 0 else
                               load_w(wlm_in[:, :, ts(nn, NV)], f"wl{nn}"))
                if not ZB:
                    lmb_t = statp.tile([1, PAIR * NV], F32, tag="lmbt", bufs=2)
                    nc.scalar.dma_start(
                        out=lmb_t[:, :],
                        in_=lmb_in[np0 * NV:(np0 + PAIR) * NV]
                        .rearrange("(o v) -> o v", o=1))
                for m in range(MT):
                    g, i = m // QT, m % QT
                    ps = psA.tile([P, Q4, NTOK], F32, tag="quad")
                    psv = ps.rearrange("p a q -> p (a q)").rearrange(
                        "p (s v) -> p s v", s=PAIR)
                    for kc in range(KC):
                        for s in range(PAIR):
                            nc.tensor.matmul(
                                out=psv[:, s, 0:NV],
                                lhsT=xfg_sb[:, g * KC + kc, ts(i, P)],
                                rhs=wls[s][:, kc, 0:NV],
                                start=(kc == 0),
                                stop=(ZB and kc == KC - 1))
                    if not ZB:
                        for s in range(PAIR):
                            nc.tensor.matmul(
                                out=psv[:, s, 0:NV],
                                lhsT=ones_bank[0:1, 0:P],
                                rhs=lmb_t[:, s * NV:(s + 1) * NV],
                                start=False, stop=True)
                    osb = outp.tile([P, PAIR, NV], BF16, tag="osb",
                                    bufs=(1 if NO_XBAR else 2))
                    nc.scalar.copy(out=osb[:, :, :], in_=psv[:, :, 0:NV])
                    (nc.scalar if STORE_ACT else nc.gpsimd).dma_start(
                        out=out_d[ts(m, P), np0 * NV:(np0 + PAIR) * NV],
                        in_=osb[:, :, :])

    nc.finalize()
    return nc


# ---------------- host side ----------------

def _bf16(a):
    return np.asarray(a, dtype=ml_dtypes.bfloat16)


def prep_inputs(cfg, inputs):
    """Build the 8 per-core input maps from full inputs (pre-tiled layouts)."""
    V, D, H, L, F, B, T = (cfg[k] for k in ("V", "D", "H", "L", "F", "B", "T"))
    NTOK = B * T // NCORES
    VS = V // NCORES
    KC = D // P
    FC = F // P

    ids = np.asarray(inputs["input_ids"]).astype(np.int32).reshape(-1)
    wte = np.asarray(inputs["wte"], dtype=np.float32)
    ln1_g = np.asarray(inputs["ln1_g"], np.float32)
    ln1_b = np.asarray(inputs["ln1_b"], np.float32)
    attn_w = np.asarray(inputs["attn_w"], np.float32)
    attn_b = np.asarray(inputs["attn_b"], np.float32)
    proj_w = np.asarray(inputs["proj_w"], np.float32)
    proj_b = np.asarray(inputs["proj_b"], np.float32)
    ln2_g = np.asarray(inputs["ln2_g"], np.float32)
    ln2_b = np.asarray(inputs["ln2_b"], np.float32)
    fc_w = np.asarray(inputs["fc_w"], np.float32)
    fc_b = np.asarray(inputs["fc_b"], np.float32)
    fc2_w = np.asarray(inputs["fc2_w"], np.float32)
    fc2_b = np.asarray(inputs["fc2_b"], np.float32)
    lnf_g = np.asarray(inputs["lnf_g"], np.float32)
    lnf_b = np.asarray(inputs["lnf_b"], np.float32)
    lm_w = np.asarray(inputs["lm_head_w"], np.float32)

    # fold LN affine into the following matmul
    wqkv_f = attn_w * ln1_g[:, None, :]                  # [L, 3D, D]
    bqkv_f = attn_b + np.einsum("lod,ld->lo", attn_w, ln1_b)
    wfc_f = fc_w * ln2_g[:, None, :]
    bfc_f = fc_b + np.einsum("lod,ld->lo", fc_w, ln2_b)
    wlm_f = lm_w * lnf_g[None, :]
    blm_f = lm_w @ lnf_b                                  # [V]

    wqkvT = wqkv_f.transpose(0, 2, 1)                     # [L, D, 3D]
    # pre-tiled [L, 3, P, KC, D] with slot order {K, V, Q}
    wq_t = wqkvT.reshape(L, KC, P, 3, D).transpose(0, 3, 2, 1, 4)
    wkvq = np.ascontiguousarray(_bf16(wq_t[:, [1, 2, 0]]))
    wprojT = np.ascontiguousarray(
        _bf16(proj_w.transpose(0, 2, 1).reshape(L, KC, P, D)
              .transpose(0, 2, 1, 3)))
    wfcT = np.ascontiguousarray(
        _bf16(wfc_f.transpose(0, 2, 1).reshape(L, KC, P, F)
              .transpose(0, 2, 1, 3)))
    wfc2T = np.ascontiguousarray(
        _bf16(fc2_w.transpose(0, 2, 1).reshape(L, FC, P, D)
              .transpose(0, 2, 1, 3)))

    qkb = np.ascontiguousarray(bqkv_f[:, :2 * D])
    vb = np.ascontiguousarray(bqkv_f[:, 2 * D:])
    wlmT_full = _bf16(wlm_f.T)                            # [D, V]

    # alibi multiplicative masks, per core, grouped by 4 heads
    slopes = 2.0 ** (-np.arange(1, H + 1, dtype=np.float64) * (ALIBI_BIAS_MAX / H))
    kk = np.arange(T, dtype=np.float64)
    HGm = min(H, 4)

    in_maps = []
    for c in range(NCORES):
        r = c % GS
        tok = ids[c * NTOK:(c + 1) * NTOK]
        qg = (r * NTOK + np.arange(NTOK, dtype=np.float64))
        rel = kk[:, None] - qg[None, :]                  # k - q_glob
        m = np.exp(slopes[:, None, None] * rel[None, :, :],
                   where=(rel[None, :, :] <= 0), out=np.zeros((H, T, NTOK)))
        m[np.broadcast_to(rel[None, :, :] > 0, m.shape)] = 0.0
        msk_t = np.ascontiguousarray(
            _bf16(m.reshape(H // HGm, HGm, T, NTOK).transpose(0, 2, 1, 3)))
        wlm_c = wlmT_full[:, c * VS:(c + 1) * VS]
        wlm_t = np.ascontiguousarray(
            wlm_c.reshape(KC, P, VS).transpose(1, 0, 2))
        in_maps.append({
            "ids": np.ascontiguousarray(tok.reshape(-1, P)),
            "wte": wte,
            "wkvq": wkvq, "wprojT": wprojT, "wfcT": wfcT, "wfc2T": wfc2T,
            "qkb": qkb, "vb": vb, "pb": np.ascontiguousarray(proj_b),
            "fcb": bfc_f, "f2b": np.ascontiguousarray(fc2_b),
            "msk": msk_t,
            "wlmT": wlm_t,
            "lmb": np.ascontiguousarray(blm_f[c * VS:(c + 1) * VS]),
        })
    return in_maps


_NC_CACHE = {}


def biases_all_zero(in_maps):
    return all(
        not np.any(np.asarray(m[k], dtype=np.float32))
        for m in in_maps for k in ("vb", "pb", "f2b", "lmb", "qkb", "fcb"))


def run(cfg, inputs):
    from concourse.bass_utils import run_bass_kernel_spmd
    in_maps = prep_inputs(cfg, inputs)
    cfg = dict(cfg, zero_bias=biases_all_zero(in_maps))
    if cfg["zero_bias"]:
        for m in in_maps:
            for k in ("qkb", "vb", "pb", "fcb", "f2b", "lmb"):
                m.pop(k)
    key = tuple(sorted(cfg.items()))
    if key not in _NC_CACHE:
        _NC_CACHE[key] = build_program(cfg)
    nc = _NC_CACHE[key]
    res = run_bass_kernel_spmd(nc, in_maps, core_ids=list(range(NCORES)))
    outs = [np.asarray(res.results[c]["out"], dtype=np.float32)
            for c in range(NCORES)]
    B, T, V = cfg["B"], cfg["T"], cfg["V"]
    logits = np.concatenate(outs, axis=1).reshape(B, T, V)
    return logits


def kernel(**inputs) -> np.ndarray:
    return run(FULL, inputs)
